# revision 10
# baseline (speedup 1.0000x reference)
"""Trainium2 Bass kernel for nn_CorrelationImage.

reference:
    corr_b = sum(map1[b] * map2[b])            # dot over C*H*W per sample
    corr   = corr / ||corr||_2                 # L2 norm over the batch
    out    = map1 + map2 * (1 - corr)[:, None, None, None]

Sharding: data-parallel over batch B=64 across 8 cores (8 samples/core).
Per core:
  1. stream the 8 (map1, map2) sample pairs into SBUF (kept resident),
  2. per-sample dot: one fused DVE tensor_tensor_reduce per sample
     (product + free-dim accumulate in a single pass, tracking the DMA
     loads), per-pair (-ones)-matmul on PE folds the partition reduce
     and the sign flip: c8n = -c_i replicated on all 128 partitions,
  3. local squares Square(c8n)=c^2 on ScalarE, 32 B AllGather of the 8
     squared dots -> every core holds all 64 squares (latency-bound:
     AG floor ~5 us vs AllReduce ~25 us observed),
  4. ss = sum of the 64 squares, replicate via K=1 ones-matmul, s_i =
     1 - c_i/sqrt(ss) = (c8n * rsqrt) + 1 in one tensor_scalar,
  5. out_i = (map2_i * s_i) + map1_i in ONE fused scalar_tensor_tensor
     per sample, alternating DVE / GpSimd, stores streamed per sample
     (first sample split in half so stores start ~1 us after s is up).
"""

import sys

import numpy as np

if "/opt/trn_rl_repo" not in sys.path:
    sys.path.insert(0, "/opt/trn_rl_repo")

B, C, H, W = 64, 64, 64, 64
N_CORES = 8
SPC = B // N_CORES  # samples per core
PART = 128
ELEMS = C * H * W  # 262144 per sample
FD = ELEMS // PART  # 2048 free-dim per sample tile

_cache = {}


def _build_nc(spc=SPC, fd=FD, n_cores=N_CORES, use_cc=True, collective="ag"):
    from contextlib import ExitStack

    from concourse import bacc, tile, mybir

    f32 = mybir.dt.float32
    Alu = mybir.AluOpType
    Act = mybir.ActivationFunctionType

    nc = bacc.Bacc(
        "TRN2", target_bir_lowering=False, debug=False, num_devices=n_cores
    )
    m1d = nc.dram_tensor("map1", [spc, PART, fd], f32, kind="ExternalInput").ap()
    m2d = nc.dram_tensor("map2", [spc, PART, fd], f32, kind="ExternalInput").ap()
    outd = nc.dram_tensor("out", [spc, PART, fd], f32, kind="ExternalOutput").ap()

    with tile.TileContext(nc) as tc, ExitStack() as ctx:
        big = ctx.enter_context(tc.tile_pool(name="big", bufs=1))
        scratch = ctx.enter_context(tc.tile_pool(name="scratch", bufs=2))
        small = ctx.enter_context(tc.tile_pool(name="small", bufs=1))
        psum = ctx.enter_context(tc.tile_pool(name="psum", bufs=1, space="PSUM"))
        dram = ctx.enter_context(tc.tile_pool(name="dram", bufs=1, space="DRAM"))

        m1s = big.tile([PART, spc * fd], f32)
        m2s = big.tile([PART, spc * fd], f32)

        # input streams first: nothing may delay the load DMAs
        for i in range(spc):
            nc.sync.dma_start(out=m1s[:, i * fd : (i + 1) * fd], in_=m1d[i])
            nc.sync.dma_start(out=m2s[:, i * fd : (i + 1) * fd], in_=m2d[i])

        # constants / warm-up, off the DMA critical path
        onesN = small.tile([PART, PART], f32)
        nc.vector.memset(onesN, -1.0)
        ones_row = small.tile([1, PART], f32)
        nc.vector.memset(ones_row, 1.0)
        partials = small.tile([PART, spc], f32)
        # preload ScalarE activation tables used later (Sqrt, Square)
        warm = small.tile([1, 1], f32)
        nc.vector.memset(warm, 1.0)
        nc.scalar.activation(out=warm, in_=warm, func=Act.Sqrt)
        nc.scalar.activation(out=warm, in_=warm, func=Act.Square)

        # per-sample dot: fused product+accumulate, one DVE pass per
        # sample so the tail after the last load byte is ~2 us; the
        # per-pair (-ones) matmuls fold partition-reduce + sign flip and
        # run on idle PE under the loads. (TensorScalarPtr with accum;
        # InstTensorTensorReduce faults this HW's ucode.)
        c8n = psum.tile([PART, spc], f32)  # -c_i replicated on partitions
        for i in range(spc):
            prod = scratch.tile([PART, fd], f32, name="prod")
            sl = slice(i * fd, (i + 1) * fd)
            nc.vector.scalar_tensor_tensor(
                out=prod,
                in0=m1s[:, sl],
                scalar=1.0,
                in1=m2s[:, sl],
                op0=Alu.bypass,
                op1=Alu.mult,
                accum_out=partials[:, i : i + 1],
            )
            if i % 2 == 1:
                nc.tensor.matmul(
                    c8n[:, i - 1 : i + 1],
                    onesN,
                    partials[:, i - 1 : i + 1],
                    start=True,
                    stop=True,
                )

        # global sum of squares across the 8 cores
        if collective == "rdma":
            # ncfw-free exchange (~5 us vs ~36 us for collective_compute):
            # Square+accum on the replicated PSUM c8n gives the local
            # sum-of-squares scalar on all 128 partitions in one ScalarE
            # op; each core then SWDGE-broadcasts that [128,1] tile into
            # slot k of the gather tile on relative peer Delta-tpb=k.
            # Receiver r's slot k holds sender r^k — a permutation of the
            # 8 cores, and only the sum matters. rsem-gated read; the
            # framework's per-kernel sem clear is fenced before the entry
            # barrier, so re-running the NEFF is safe.
            sq8 = small.tile([PART, spc], f32)
            cls = small.tile([PART, 1], f32)
            nc.scalar.activation(out=sq8, in_=c8n, func=Act.Square, accum_out=cls)
            gather = small.tile([PART, n_cores], f32)
            rsem = nc.alloc_semaphore("rdma_gather_sem")
            lsem = nc.alloc_semaphore("rdma_local_sem")
            psem = nc.alloc_semaphore("rdma_prep_sem")
            with tc.tile_critical(name="rdma_xchg"):
                nc.gpsimd.bir_kernel_barrier_wait([list(range(n_cores))])
                for k in range(n_cores):
                    rdests = [(0, k) if j == k else None for j in range(n_cores)]
                    nc.gpsimd.remote_dma_broadcast(
                        out_ap=gather[:, k : k + 1],
                        in_ap=cls,
                        remote_sem=rsem,
                        local_sem=lsem,
                        rdests=rdests,
                    ).then_inc(psem, 1)
                nc.gpsimd.wait_ge(psem, n_cores)
                nc.gpsimd.trigger_dma(count=n_cores)
                nc.gpsimd.wait_ge(rsem, 2 * n_cores)
            ssb = small.tile([PART, 1], f32)
            nc.vector.tensor_reduce(
                out=ssb, in_=gather, axis=mybir.AxisListType.X, op=Alu.add
            )
            norm_src = ssb
        else:
            csq = small.tile([1, spc], f32)
            nc.scalar.activation(out=csq, in_=c8n[0:1, :], func=Act.Square)
            if use_cc and collective == "ag":
                sq_all = small.tile([1, n_cores * spc], f32)
                cc_in = dram.tile([spc], f32)
                nc.sync.dma_start(out=cc_in[:], in_=csq[:])
                cc_out = dram.tile([n_cores * spc], f32, addr_space="Shared")
                nc.gpsimd.collective_compute(
                    "AllGather",
                    Alu.bypass,
                    replica_groups=[list(range(n_cores))],
                    ins=[cc_in.opt()],
                    outs=[cc_out.opt()],
                )
                nc.sync.dma_start(out=sq_all[:], in_=cc_out[:])
            elif use_cc:
                sq_all = small.tile([1, spc], f32)
                cc_in = dram.tile([spc], f32)
                nc.sync.dma_start(out=cc_in[:], in_=csq[:])
                cc_out = dram.tile([spc], f32, addr_space="Shared")
                nc.gpsimd.collective_compute(
                    "AllReduce",
                    Alu.add,
                    replica_groups=[list(range(n_cores))],
                    ins=[cc_in.opt()],
                    outs=[cc_out.opt()],
                )
                nc.sync.dma_start(out=sq_all[:], in_=cc_out[:])
            else:
                # debug only: pretend every core holds the same 8 samples
                sq_all = small.tile([1, spc], f32)
                nc.vector.tensor_scalar_mul(
                    out=sq_all, in0=csq, scalar1=float(n_cores)
                )

            ss = small.tile([1, 1], f32)
            nc.vector.tensor_reduce(
                out=ss, in_=sq_all, axis=mybir.AxisListType.X, op=Alu.add
            )
            # replicate ss across partitions via a K=1 ones-matmul
            ssp = psum.tile([PART, 1], f32)
            nc.tensor.matmul(ssp, ones_row, ss, start=True, stop=True)
            norm_src = ssp

        # s_i = (-c_i) * rsqrt(ss) + 1 on all partitions in one op
        normb = small.tile([PART, 1], f32)
        nc.scalar.activation(out=normb, in_=norm_src, func=Act.Sqrt)
        inv = small.tile([PART, 1], f32)
        nc.vector.reciprocal(out=inv, in_=normb)
        s8 = small.tile([PART, spc], f32)
        nc.vector.tensor_scalar(
            out=s8,
            in0=c8n,
            scalar1=inv,
            scalar2=1.0,
            op0=Alu.mult,
            op1=Alu.add,
        )

        # out_i = (map2_i * s_i) + map1_i fully fused, in place in the
        # map2 buffer, one DVE op per sample (2.2 us < 2.9 us store
        # pace, so DVE alone keeps the store stream fed; neuronxcc
        # rejects TensorScalarPtr on Pool). First sample computed in
        # halves so the store stream starts ~1 us after s8.
        def stt(lo, hi, i):
            nc.vector.scalar_tensor_tensor(
                out=m2s[:, lo:hi],
                in0=m2s[:, lo:hi],
                scalar=s8[:, i : i + 1],
                in1=m1s[:, lo:hi],
                op0=Alu.mult,
                op1=Alu.add,
            )

        half = fd // 2
        stt(0, half, 0)
        nc.sync.dma_start(out=outd[0][:, 0:half], in_=m2s[:, 0:half])
        stt(half, fd, 0)
        nc.sync.dma_start(out=outd[0][:, half:fd], in_=m2s[:, half:fd])
        for i in range(1, spc):
            sl = slice(i * fd, (i + 1) * fd)
            stt(i * fd, (i + 1) * fd, i)
            nc.sync.dma_start(out=outd[i], in_=m2s[:, sl])

    nc.compile()
    return nc


import os

COLLECTIVE = os.environ.get("KERNEL_COLLECTIVE", "rdma")


def _get_nc():
    if "nc" not in _cache:
        _cache["nc"] = _build_nc(collective=COLLECTIVE)
    return _cache["nc"]


def kernel(map1, map2):
    from concourse.bass_utils import run_bass_kernel_spmd

    nc = _get_nc()
    m1 = np.ascontiguousarray(np.asarray(map1, dtype=np.float32)).reshape(
        N_CORES, SPC, PART, FD
    )
    m2 = np.ascontiguousarray(np.asarray(map2, dtype=np.float32)).reshape(
        N_CORES, SPC, PART, FD
    )
    in_maps = [{"map1": m1[c], "map2": m2[c]} for c in range(N_CORES)]
    res = run_bass_kernel_spmd(nc, in_maps, list(range(N_CORES)))
    out = np.concatenate(
        [res.results[c]["out"].reshape(SPC, C, H, W) for c in range(N_CORES)],
        axis=0,
    )
    return out
